# revision 2
# baseline (speedup 1.0000x reference)
"""Trainium2 Bass kernel for nn_Attention_71966472012100.

Multi-head attention, B=4, S=2048, H=12, D=100, HID=1200, bug-faithful
head-mixing reshape before the output projection.

Sharding: 8 cores = batch (4) x head-group (2 groups of 6 heads). Each core
produces 1024 complete rows of the final output; no cross-core comms.

v2 changes vs baseline (all aimed at keeping PE busy):
  - V': xt streamed in tg-half chunks; wv loaded as one bulk tile;
    PSUM quarter-groups (4 banks) so evictions overlap the next group.
  - Q/K: bulk per-head weight tiles; 2-bank PSUM groups (q/k x s-half);
    head 0 kept in SBUF, heads 1-5 spilled to DRAM and read back per-head
    during attention (rotating qt/kt pool, prefetched one head ahead).
  - Softmax normalization fully on-chip (no DRAM round-trip): po evicted to
    an f32r scratch, sums row extracted with a unit-vector matmul, DVE
    reciprocal, ones-matmul broadcast, normalize fused into the ot eviction.
  - ot and wo are bf16: halves their SBUF footprint so all of wo stays
    resident, letting WO chains interleave into the attention phase (fills
    the PE bubbles left by the ACT-bound exp stream).
  - WO chains (rt, jb) are emitted as soon as their heads are normalized.
"""

import numpy as np
from contextlib import ExitStack

import ml_dtypes
import concourse.bass as bass
import concourse.tile as tile
from concourse import bacc
from concourse import mybir
from concourse.bass_utils import run_bass_kernel_spmd

F32 = mybir.dt.float32
F32R = mybir.dt.float32r
BF16 = mybir.dt.bfloat16
EXP = mybir.ActivationFunctionType.Exp

B, S, H, D, HID = 4, 2048, 12, 100, 1200
HG = 2                # head groups (tensor parallel)
HL = H // HG          # 6 heads per core
ROWS = S * HL * D // HID   # 1024 output rows per core
CK, CCH = 120, 10     # contraction chunking of HID
TT = S // 128         # 16 key tiles
VW = HL * D + HL      # 606: V' row width per t-tile (d cols + ones col per head)
NM = HID // D         # 12 m-chunks in the output projection

# rt -> last head needed (columns [rt*1536,(rt+1)*1536) of ot, heads are S wide)
RT_LAST_HEAD = [((rt + 1) * 1536 - 1) // S for rt in range(8)]


def _mm(nc, out, lhsT, rhs, **kw):
    nc.tensor.matmul(out, lhsT.bitcast(F32R), rhs.bitcast(F32R), **kw)


def _absorb(nc, ap):
    """PE-side observation of a freshly DMA'd tile (absorbs a DMA wait)."""
    bb = ap.bitcast(BF16)
    nc.tensor.ldweights(bb[:, 0:1])


def build_program(scale: float, n_iters: int = 1):
    nc = bacc.Bacc("TRN2", target_bir_lowering=False, debug=False)

    tn = {}
    tn["xT"] = nc.dram_tensor("xT", [HID, S], F32R, kind="ExternalInput")
    tn["wqT"] = nc.dram_tensor("wqT", [HID, HL * D], F32R, kind="ExternalInput")
    tn["wkT"] = nc.dram_tensor("wkT", [HID, HL * D], F32R, kind="ExternalInput")
    tn["wvT"] = nc.dram_tensor("wvT", [HID, HL * D], F32R, kind="ExternalInput")
    tn["woT"] = nc.dram_tensor("woT", [HID, HID], BF16, kind="ExternalInput")
    tn["biasT"] = nc.dram_tensor("biasT", [128, HL * TT], F32, kind="ExternalInput")
    tn["y"] = nc.dram_tensor("y", [ROWS, HID], F32, kind="ExternalOutput")
    tn["qsp"] = nc.dram_tensor("q_spill", [(HL - 1) * D, S], F32R)
    tn["ksp"] = nc.dram_tensor("k_spill", [(HL - 1) * D, S], F32R)

    with tile.TileContext(nc) as tc:
        for _ in range(n_iters):
            _emit_iter(nc, tc, tn, scale)
    nc.compile()
    return nc


def _emit_iter(nc, tc, tn, scale):
    xT, wqT, wkT, wvT, woT = tn["xT"], tn["wqT"], tn["wkT"], tn["wvT"], tn["woT"]
    biasT, y, qsp, ksp = tn["biasT"], tn["y"], tn["qsp"], tn["ksp"]

    with ExitStack() as ctx:
        pa = ctx.enter_context(tc.tile_pool(name="pa", bufs=1))
        vp = pa.tile([128, TT * VW], F32R, name="vp")  # col = tt*VW + h*101 + d
        bias_sb = pa.tile([128, HL * TT], F32, name="bias_sb")
        ones1 = pa.tile([1, D], F32R, name="ones1")
        nc.sync.dma_start(out=bias_sb, in_=biasT.ap())
        nc.vector.memset(ones1.bitcast(F32), 1.0)
        nc.vector.tensor_copy(out=ones1, in_=ones1.bitcast(F32))

        # rotating q/k SBUF tiles: head 0 written by P1, 1..5 read back
        pqk = ctx.enter_context(tc.tile_pool(name="pqk", bufs=1))
        qt = {}
        kt = {}

        def qk_tiles(h):
            qt[h] = pqk.tile([D, S], F32R, tag="qt", name=f"qt{h}", bufs=3)
            kt[h] = pqk.tile([D, S], F32R, tag="kt", name=f"kt{h}", bufs=3)

        # ================= P1: V' + Q/K projections =======================
        with tc.tile_pool(name="pxt", bufs=1) as pxt, \
             tc.tile_pool(name="pwqk", bufs=1) as pwqk:
            xt = pxt.tile([CK, CCH * S], F32R, name="xt")

            # ones cols pre-set; V cols overwritten
            nc.vector.memset(vp.bitcast(F32), 1.0)
            ones_v = vp.rearrange("p (n k) -> p n k", k=101)[:, :, 100]
            nc.vector.tensor_copy(out=ones_v, in_=ones_v.bitcast(F32))

            def wqk_tiles(h):
                wqh = pwqk.tile([CK, CCH * D], F32R, tag="wq",
                                name=f"wq{h}", bufs=2)
                wkh = pwqk.tile([CK, CCH * D], F32R, tag="wk",
                                name=f"wk{h}", bufs=2)
                for wtile, wdram in ((wqh, wqT), (wkh, wkT)):
                    nc.sync.dma_start(
                        out=wtile.rearrange("p (c j) -> p c j", j=D),
                        in_=wdram.ap()[:, h * D : (h + 1) * D]
                        .rearrange("(c p) j -> p c j", p=CK))
                return wqh, wkh

            # ---- V' ----
            with tc.tile_pool(name="psv", bufs=8, space="PSUM") as psv, \
                 tc.tile_pool(name="pwv", bufs=1) as pwv:
                # wv bulk, loaded per chunk: col = c*600 + jh*300 + v
                wvf = pwv.tile([CK, CCH * 2 * 300], F32R, name="wvf")
                wqk0 = None
                # groups: (tg, jh, tile list); last group split in two so the
                # QK section's PSUM allocs only wait on a 4-bank eviction
                groups = [(0, 0, range(8)), (0, 1, range(8)), (1, 0, range(8)),
                          (1, 1, range(4)), (1, 1, range(4, 8))]
                for gi, (tg, jh, tiles) in enumerate(groups):
                    accs = {i: psv.tile([128, 300], F32, tag="vacc",
                                        name="vacc") for i in tiles}
                    for c in range(CCH):
                        if gi == 0:
                            # stream first x half + wv chunk by chunk
                            nc.sync.dma_start(
                                out=xt[:, c * S : c * S + 1024],
                                in_=xT.ap()[c * CK : (c + 1) * CK, 0:1024])
                            _absorb(nc, xt[:, c * S : c * S + 1024])
                            nc.sync.dma_start(
                                out=wvf[:, c * 600 : (c + 1) * 600],
                                in_=wvT.ap()[c * CK : (c + 1) * CK, :])
                            _absorb(nc, wvf[:, c * 600 : (c + 1) * 600])
                        if gi == 1:
                            # prefetch second x half (used by tg=1 groups)
                            nc.sync.dma_start(
                                out=xt[:, c * S + 1024 : c * S + 2048],
                                in_=xT.ap()[c * CK : (c + 1) * CK,
                                            1024:2048])
                        if gi == 2:
                            _absorb(nc, xt[:, c * S + 1024 : c * S + 2048])
                        for i in tiles:
                            col = c * S + tg * 1024 + i * 128
                            _mm(nc, accs[i][:, :],
                                xt[:, col : col + 128],
                                wvf[:, c * 600 + jh * 300 :
                                    c * 600 + (jh + 1) * 300],
                                start=(c == 0), stop=(c == CCH - 1))
                    if gi == 0:
                        # prefetch head 0 projection weights
                        wqk0 = wqk_tiles(0)
                    for i in tiles:
                        tt = tg * 8 + i
                        for hh in range(3):
                            h = jh * 3 + hh
                            c0 = tt * VW + h * 101
                            nc.vector.tensor_copy(
                                out=vp[:, c0 : c0 + D],
                                in_=accs[i][:, hh * D : (hh + 1) * D])

            # ---- Q/K: all 6 heads; head 0 resident, 1-5 spilled ----
            with tc.tile_pool(name="pstg", bufs=2) as pstg, \
                 tc.tile_pool(name="psq", bufs=4, space="PSUM") as psq:
                for h in range(HL):
                    if h == 0:
                        wqh, wkh = wqk0
                        qk_tiles(0)
                    else:
                        wqh, wkh = wqk_tiles(h)
                    _absorb(nc, wqh)
                    _absorb(nc, wkh)
                    for wtile, dest_sb, dest_dram in (
                            (wqh, qt.get(h), qsp), (wkh, kt.get(h), ksp)):
                        for sh in range(2):
                            acc = psq.tile([D, 1024], F32, tag="qkacc",
                                           name="qkacc")
                            for c in range(CCH):
                                for sb in range(2):
                                    s0 = c * S + sh * 1024 + sb * 512
                                    _mm(nc, acc[:, sb * 512 : (sb + 1) * 512],
                                        wtile[:, c * D : (c + 1) * D],
                                        xt[:, s0 : s0 + 512],
                                        start=(c == 0), stop=(c == CCH - 1))
                            if h == 0:
                                nc.vector.tensor_copy(
                                    out=dest_sb[:, sh * 1024 : (sh + 1) * 1024],
                                    in_=acc[:, :])
                            else:
                                stg = pstg.tile([D, 1024], F32R, tag="stg",
                                                name="stg")
                                nc.vector.tensor_copy(out=stg, in_=acc[:, :])
                                nc.sync.dma_start(
                                    out=dest_dram.ap()[
                                        (h - 1) * D : h * D,
                                        sh * 1024 : (sh + 1) * 1024],
                                    in_=stg)

        # ================= attention + interleaved WO =====================
        with tc.tile_pool(name="pat", bufs=1, side="right") as pat:
            ot = pat.tile([D, HL * S], BF16, name="ot")
            wob = pat.tile([D, NM * HID], BF16, name="wob")  # col = m*HID + ycol
            for m in range(NM):
                nc.sync.dma_start(
                    out=wob[:, m * HID : (m + 1) * HID],
                    in_=woT.ap()[m * D : (m + 1) * D, :])
            ot_r = ot.rearrange("p (r m) -> p r m", m=NM)

            with tc.tile_pool(name="psa", bufs=2, space="PSUM") as psa, \
                 tc.tile_pool(name="pso", bufs=1, space="PSUM") as pso, \
                 tc.tile_pool(name="psw", bufs=2, space="PSUM") as psw, \
                 tc.tile_pool(name="ppt", bufs=3) as ppt, \
                 tc.tile_pool(name="pnr", bufs=2) as pnr, \
                 tc.tile_pool(name="por", bufs=2) as por, \
                 tc.tile_pool(name="pyb", bufs=3) as pyb:

                # WO slot machine: one matmul per call, interleaved into the
                # attention tt loops so the PE always has exp-independent work
                wo_st = {"queue": [], "chain": None, "m": 0, "pys": None}

                def wo_slot(n=1):
                    for _ in range(n):
                        if wo_st["chain"] is None:
                            if not wo_st["queue"]:
                                return
                            wo_st["chain"] = wo_st["queue"].pop(0)
                            wo_st["m"] = 0
                            wo_st["pys"] = psw.tile([128, 512], F32, tag="pys",
                                                    name="pys", bufs=1)
                        rt, jb = wo_st["chain"]
                        m = wo_st["m"]
                        nc.tensor.matmul(
                            wo_st["pys"][:, 0:400],
                            ot_r[:, rt * 128 : (rt + 1) * 128, m],
                            wob[:, m * HID + jb * 400 :
                                m * HID + (jb + 1) * 400],
                            start=(m == 0), stop=(m == NM - 1))
                        wo_st["m"] += 1
                        if wo_st["m"] == NM:
                            ysb = pyb.tile([128, 400], F32, tag="ysb",
                                           name="ysb")
                            nc.vector.tensor_copy(out=ysb,
                                                  in_=wo_st["pys"][:, 0:400])
                            nc.sync.dma_start(
                                out=y.ap()[rt * 128 : (rt + 1) * 128,
                                           jb * 400 : (jb + 1) * 400],
                                in_=ysb)
                            wo_st["chain"] = None

                for h in range(HL):
                    if 1 <= h + 1 < HL:
                        # prefetch next head's q/k from DRAM spill
                        qk_tiles(h + 1)
                        for dst, src in ((qt[h + 1], qsp), (kt[h + 1], ksp)):
                            nc.sync.dma_start(
                                out=dst, in_=src.ap()[h * D : (h + 1) * D, :])
                            _absorb(nc, dst)
                    for sh in range(2):
                        s0 = sh * 1024
                        po = pso.tile([D + 1, 1024], F32, tag="po", name="po")
                        # software-pipelined: PV(tt-1) emitted after scores(tt)
                        pts = [None] * TT
                        for tt in range(TT + 1):
                            if tt < TT:
                                ss = psa.tile([128, 1024], F32, tag="ss",
                                              name="ss")
                                for sbb in range(2):
                                    _mm(nc, ss[:, sbb * 512 : (sbb + 1) * 512],
                                        kt[h][:, tt * 128 : (tt + 1) * 128],
                                        qt[h][:, s0 + sbb * 512 :
                                              s0 + (sbb + 1) * 512],
                                        start=True, stop=True)
                                wo_slot(1)
                                pt = ppt.tile([128, 1024], F32R, tag="pt",
                                              name="pt")
                                nc.scalar.activation(
                                    out=pt, in_=ss[:, :], func=EXP,
                                    bias=bias_sb[:, h * TT + tt :
                                                 h * TT + tt + 1],
                                    scale=scale)
                                pts[tt] = pt
                            if tt >= 1:
                                for sbb in range(2):
                                    _mm(nc, po[:, sbb * 512 : (sbb + 1) * 512],
                                        vp[:, (tt - 1) * VW + h * 101 :
                                           (tt - 1) * VW + h * 101 + 101],
                                        pts[tt - 1][:, sbb * 512 :
                                                    (sbb + 1) * 512],
                                        start=(tt == 1), stop=(tt == TT))
                                pts[tt - 1] = None
                                wo_slot(1)
                        # ---- on-chip normalization + eviction to ot ----
                        orow = por.tile([D + 1, 1024], F32R, tag="orow",
                                        name="orow")
                        nc.vector.tensor_copy(out=orow, in_=po[:, :])
                        # sums row (partition 100) -> partition 0 via DMA
                        srow = pnr.tile([1, 1024], F32R, tag="srow",
                                        name="srow")
                        rrow = pnr.tile([1, 1024], F32R, tag="rrow", name="rrow")
                        nc.sync.dma_start(out=srow, in_=orow[D : D + 1, :])
                        with nc.allow_low_precision(reason="softmax recip"):
                            nc.vector.reciprocal(out=rrow, in_=srow)
                        for blk in range(2):
                            pb = psw.tile([128, 512], F32, tag="nrm",
                                          name="pb", bufs=1)
                            _mm(nc, pb[0:D, :], ones1[0:1, :],
                                rrow[0:1, blk * 512 : (blk + 1) * 512],
                                start=True, stop=True)
                            nc.vector.tensor_mul(
                                ot[:, h * S + s0 + blk * 512 :
                                   h * S + s0 + (blk + 1) * 512],
                                orow[0:D, blk * 512 : (blk + 1) * 512],
                                pb[0:D, :])
                    # head h normalized -> unlock rts
                    for rt in range(8):
                        if RT_LAST_HEAD[rt] == h:
                            for jb in range(3):
                                wo_st["queue"].append((rt, jb))
                while wo_st["queue"] or wo_st["chain"] is not None:
                    wo_slot(1)


def make_core_inputs(x, alibi, attention_mask, wq, wk, wv, wo, layer_index):
    li = int(np.asarray(layer_index))
    inv = np.float32(1.0 / (li + 1))
    woT = np.ascontiguousarray(
        np.asarray(wo, dtype=np.float32).T).astype(ml_dtypes.bfloat16)
    xTs = [np.ascontiguousarray(np.asarray(x[b], dtype=np.float32).T)
           for b in range(B)]
    wts = []
    for g in range(HG):
        sl = slice(g * HL * D, (g + 1) * HL * D)
        wts.append(tuple(
            np.ascontiguousarray(np.asarray(w, dtype=np.float32)[sl, :].T)
            for w in (wq, wk, wv)))
    in_maps = []
    for b in range(B):
        for g in range(HG):
            a = np.asarray(alibi, dtype=np.float32)[
                b * H + g * HL : b * H + (g + 1) * HL, 0, :]      # (6, S)
            msk = np.asarray(attention_mask, dtype=np.float32)[b, 0, 0, :S]
            bias = a * inv + msk[None, :]                          # (6, S)
            biasT = np.ascontiguousarray(
                bias.reshape(HL, TT, 128).transpose(2, 0, 1).reshape(128, HL * TT))
            wqT, wkT, wvT = wts[g]
            in_maps.append({
                "xT": xTs[b], "wqT": wqT, "wkT": wkT, "wvT": wvT,
                "woT": woT, "biasT": biasT,
            })
    scale = float(np.float32(np.sqrt(np.float32(D))) * inv)
    return in_maps, scale


def run(trace=False, **inputs):
    in_maps, scale = make_core_inputs(**inputs)
    nc = build_program(scale)
    res = run_bass_kernel_spmd(nc, in_maps, core_ids=list(range(B * HG)),
                               trace=trace)
    out = np.empty((B, S, HID), dtype=np.float32)
    for b in range(B):
        for g in range(HG):
            out[b, g * ROWS : (g + 1) * ROWS, :] = res.results[b * HG + g]["y"]
    return out, res


def kernel(**inputs) -> np.ndarray:
    out, _ = run(trace=False, **inputs)
    return out


# revision 12
# speedup vs baseline: 1.2701x; 1.2701x over previous
"""Trainium2 Bass kernel for nn_Attention_71966472012100.

Multi-head attention, B=4, S=2048, H=12, D=100, HID=1200, bug-faithful
head-mixing reshape before the output projection.

Sharding: 8 cores = batch (4) x head-group (2 groups of 6 heads). Each core
produces 1024 complete rows of the final output; no cross-core comms.

v2 changes vs baseline (all aimed at keeping PE busy):
  - V': xt streamed in tg-half chunks; wv loaded as one bulk tile;
    PSUM quarter-groups (4 banks) so evictions overlap the next group.
  - Q/K: bulk per-head weight tiles; 2-bank PSUM groups (q/k x s-half);
    head 0 kept in SBUF, heads 1-5 spilled to DRAM and read back per-head
    during attention (rotating qt/kt pool, prefetched one head ahead).
  - Softmax normalization fully on-chip (no DRAM round-trip): po evicted to
    an f32r scratch, sums row extracted with a unit-vector matmul, DVE
    reciprocal, ones-matmul broadcast, normalize fused into the ot eviction.
  - ot and wo are bf16: halves their SBUF footprint so all of wo stays
    resident, letting WO chains interleave into the attention phase (fills
    the PE bubbles left by the ACT-bound exp stream).
  - WO chains (rt, jb) are emitted as soon as their heads are normalized.
"""

import numpy as np
from contextlib import ExitStack

import ml_dtypes
import concourse.bass as bass
import concourse.tile as tile
from concourse import bacc
from concourse import mybir
from concourse.bass_utils import run_bass_kernel_spmd

F32 = mybir.dt.float32
F32R = mybir.dt.float32r
BF16 = mybir.dt.bfloat16
EXP = mybir.ActivationFunctionType.Exp

B, S, H, D, HID = 4, 2048, 12, 100, 1200
HG = 2                # head groups (tensor parallel)
HL = H // HG          # 6 heads per core
ROWS = S * HL * D // HID   # 1024 output rows per core
CK, CCH = 120, 10     # contraction chunking of HID
TT = S // 128         # 16 key tiles
VW = HL * D + HL      # 606: V' row width per t-tile (d cols + ones col per head)
NM = HID // D         # 12 m-chunks in the output projection

# rt -> last head needed (columns [rt*1536,(rt+1)*1536) of ot, heads are S wide)
RT_LAST_HEAD = [((rt + 1) * 1536 - 1) // S for rt in range(8)]


def _mm(nc, out, lhsT, rhs, **kw):
    nc.tensor.matmul(out, lhsT.bitcast(F32R), rhs.bitcast(F32R), **kw)


def _absorb(nc, ap):
    """PE-side observation of a freshly DMA'd tile (absorbs a DMA wait)."""
    bb = ap.bitcast(BF16)
    nc.tensor.ldweights(bb[:, 0:1])


def build_program(scale: float, n_iters: int = 1):
    nc = bacc.Bacc("TRN2", target_bir_lowering=False, debug=False)

    tn = {}
    tn["xT"] = nc.dram_tensor("xT", [HID, S], F32R, kind="ExternalInput")
    tn["wqT"] = nc.dram_tensor("wqT", [HID, HL * D], F32R, kind="ExternalInput")
    tn["wkT"] = nc.dram_tensor("wkT", [HID, HL * D], F32R, kind="ExternalInput")
    tn["wvT"] = nc.dram_tensor("wvT", [HID, HL * D], F32R, kind="ExternalInput")
    tn["woT"] = nc.dram_tensor("woT", [HID, HID], BF16, kind="ExternalInput")
    tn["biasT"] = nc.dram_tensor("biasT", [128, HL * TT], F32, kind="ExternalInput")
    tn["y"] = nc.dram_tensor("y", [ROWS, HID], F32, kind="ExternalOutput")
    tn["qsp"] = nc.dram_tensor("q_spill", [(HL - 1) * D, S], F32R)
    tn["ksp"] = nc.dram_tensor("k_spill", [(HL - 1) * D, S], F32R)

    with tile.TileContext(nc) as tc:
        for _ in range(n_iters):
            _emit_iter(nc, tc, tn, scale)
    nc.compile()
    return nc


def _emit_iter(nc, tc, tn, scale):
    xT, wqT, wkT, wvT, woT = tn["xT"], tn["wqT"], tn["wkT"], tn["wvT"], tn["woT"]
    biasT, y, qsp, ksp = tn["biasT"], tn["y"], tn["qsp"], tn["ksp"]

    with ExitStack() as ctx:
        pa = ctx.enter_context(tc.tile_pool(name="pa", bufs=1))
        vp = pa.tile([128, TT * VW], F32R, name="vp")  # col = tt*VW + h*101 + d
        bias_sb = pa.tile([128, HL * TT], F32, name="bias_sb")
        ones1 = pa.tile([1, D], F32R, name="ones1")
        nc.vector.memset(ones1.bitcast(F32), 1.0)
        nc.vector.tensor_copy(out=ones1, in_=ones1.bitcast(F32))
        # pre-warm the ACT exp table so the first real exp doesn't pay the load
        warm = pa.tile([1, 4], F32R, name="warm")
        nc.scalar.activation(out=warm, in_=ones1[:, 0:4], func=EXP, scale=1.0)

        # rotating q/k SBUF tiles: head 0 written by P1, 1..5 read back
        pqk = ctx.enter_context(tc.tile_pool(name="pqk", bufs=1))
        qt = {}
        kt = {}

        def qk_tiles(h):
            qt[h] = pqk.tile([D, S], F32R, tag="qt", name=f"qt{h}", bufs=3)
            kt[h] = pqk.tile([D, S], F32R, tag="kt", name=f"kt{h}", bufs=3)

        # ================= P1: V' + Q/K projections =======================
        with tc.tile_pool(name="pxt", bufs=1) as pxt, \
             tc.tile_pool(name="pwqk", bufs=1) as pwqk:
            xt = pxt.tile([CK, CCH * S], F32R, name="xt")

            # ones cols pre-set; V cols overwritten
            nc.vector.memset(vp.bitcast(F32), 1.0)
            ones_v = vp.rearrange("p (n k) -> p n k", k=101)[:, :, 100]
            nc.vector.tensor_copy(out=ones_v, in_=ones_v.bitcast(F32))

            def wqk_tiles(h):
                wqh = pwqk.tile([CK, CCH * D], F32R, tag="wq",
                                name=f"wq{h}", bufs=2)
                wkh = pwqk.tile([CK, CCH * D], F32R, tag="wk",
                                name=f"wk{h}", bufs=2)
                for wtile, wdram in ((wqh, wqT), (wkh, wkT)):
                    nc.sync.dma_start(
                        out=wtile.rearrange("p (c j) -> p c j", j=D),
                        in_=wdram.ap()[:, h * D : (h + 1) * D]
                        .rearrange("(c p) j -> p c j", p=CK))
                return wqh, wkh

            # ---- V' ----
            with tc.tile_pool(name="psv", bufs=8, space="PSUM") as psv, \
                 tc.tile_pool(name="pwv", bufs=1) as pwv:
                # wv bulk, loaded per chunk: col = c*600 + jh*300 + v
                wvf = pwv.tile([CK, CCH * 2 * 300], F32R, name="wvf")
                wqk0 = None
                # 4 groups of 4 t-tiles x both jh halves (8 one-bank accs):
                # each group consumes only 512 xt cols per chunk, so the x
                # stream (0.79us/chunk) stays ahead of the PE (1us/chunk)
                def xt_piece(nc_, g, c):
                    nc_.sync.dma_start(
                        out=xt[:, c * S + g * 512 : c * S + (g + 1) * 512],
                        in_=xT.ap()[c * CK : (c + 1) * CK,
                                    g * 512 : (g + 1) * 512])
                vgroups = [range(0, 4), range(4, 8), range(8, 12),
                           range(12, 14), range(14, 16)]
                for g, tset in enumerate(vgroups):
                    accs = {(i, jh): psv.tile([128, 300], F32, tag="vacc",
                                              name="vacc")
                            for i in tset for jh in range(2)}
                    for c in range(CCH):
                        if g == 0:
                            # own pieces + next group's + the wv chunk
                            xt_piece(nc, 0, c)
                            _absorb(nc, xt[:, c * S : c * S + 512])
                            nc.sync.dma_start(
                                out=wvf[:, c * 600 : (c + 1) * 600],
                                in_=wvT.ap()[c * CK : (c + 1) * CK, :])
                            _absorb(nc, wvf[:, c * 600 : (c + 1) * 600])
                            xt_piece(nc, 1, c)
                        elif g < 3:
                            xt_piece(nc, g + 1, c)
                        if g >= 1:
                            p0, p1 = tset[0] * 128, tset[-1] * 128 + 128
                            _absorb(nc, xt[:, c * S + p0 : c * S + p1])
                        for i in tset:
                            col = c * S + i * 128
                            for jh in range(2):
                                _mm(nc, accs[i, jh][:, :],
                                    xt[:, col : col + 128],
                                    wvf[:, c * 600 + jh * 300 :
                                        c * 600 + (jh + 1) * 300],
                                    start=(c == 0), stop=(c == CCH - 1))
                    if g == 0:
                        # prefetch head 0 projection weights + exp bias
                        wqk0 = wqk_tiles(0)
                        nc.sync.dma_start(out=bias_sb, in_=biasT.ap())
                    for i in tset:
                        for jh in range(2):
                            c0 = i * VW + jh * 3 * 101
                            dst = vp[:, c0 : c0 + 3 * 101].rearrange(
                                "p (hh k) -> p hh k", k=101)[:, :, 0:D]
                            nc.vector.tensor_copy(
                                out=dst,
                                in_=accs[i, jh].rearrange(
                                    "p (hh k) -> p hh k", k=D))

            # ---- Q/K: all 6 heads; head 0 resident, 1-5 spilled ----
            with tc.tile_pool(name="pstg", bufs=2) as pstg, \
                 tc.tile_pool(name="psq", bufs=4, space="PSUM") as psq:
                for h in range(HL):
                    if h == 0:
                        wqh, wkh = wqk0
                        qk_tiles(0)
                    else:
                        wqh, wkh = wqk_tiles(h)
                    _absorb(nc, wqh)
                    _absorb(nc, wkh)
                    for wtile, dest_sb, dest_dram in (
                            (wqh, qt.get(h), qsp), (wkh, kt.get(h), ksp)):
                        for sh in range(2):
                            acc = psq.tile([D, 1024], F32, tag="qkacc",
                                           name="qkacc")
                            for c in range(CCH):
                                for sb in range(2):
                                    s0 = c * S + sh * 1024 + sb * 512
                                    _mm(nc, acc[:, sb * 512 : (sb + 1) * 512],
                                        wtile[:, c * D : (c + 1) * D],
                                        xt[:, s0 : s0 + 512],
                                        start=(c == 0), stop=(c == CCH - 1))
                            if h == 0:
                                nc.vector.tensor_copy(
                                    out=dest_sb[:, sh * 1024 : (sh + 1) * 1024],
                                    in_=acc[:, :])
                            else:
                                stg = pstg.tile([D, 1024], F32R, tag="stg",
                                                name="stg")
                                nc.vector.tensor_copy(out=stg, in_=acc[:, :])
                                nc.sync.dma_start(
                                    out=dest_dram.ap()[
                                        (h - 1) * D : h * D,
                                        sh * 1024 : (sh + 1) * 1024],
                                    in_=stg)

        # ================= attention + interleaved WO =====================
        with tc.tile_pool(name="pat", bufs=1, side="right") as pat:
            ot = pat.tile([D, HL * S], BF16, name="ot")
            wob = pat.tile([D, NM * HID], BF16, name="wob")  # col = m*HID + ycol
            for m in range(NM):
                nc.sync.dma_start(
                    out=wob[:, m * HID : (m + 1) * HID],
                    in_=woT.ap()[m * D : (m + 1) * D, :])
            ot_r = ot.rearrange("p (r m) -> p r m", m=NM)

            with tc.tile_pool(name="psa", bufs=2, space="PSUM") as psa, \
                 tc.tile_pool(name="pso", bufs=1, space="PSUM") as pso, \
                 tc.tile_pool(name="psw", bufs=2, space="PSUM") as psw, \
                 tc.tile_pool(name="ppt", bufs=3) as ppt, \
                 tc.tile_pool(name="pnr", bufs=2) as pnr, \
                 tc.tile_pool(name="por", bufs=2) as por, \
                 tc.tile_pool(name="pyb", bufs=3) as pyb:

                # WO slot machine: one matmul per call, interleaved into the
                # attention tt loops so the PE always has exp-independent work
                wo_st = {"queue": [], "chain": None, "m": 0, "pys": None}

                def wo_slot(n=1):
                    for _ in range(n):
                        if wo_st["chain"] is None:
                            if not wo_st["queue"]:
                                return
                            wo_st["chain"] = wo_st["queue"].pop(0)
                            wo_st["m"] = 0
                            wo_st["pys"] = psw.tile([128, 512], F32, tag="pys",
                                                    name="pys", bufs=1)
                        rt, jb = wo_st["chain"]
                        m = wo_st["m"]
                        nc.tensor.matmul(
                            wo_st["pys"][:, 0:400],
                            ot_r[:, rt * 128 : (rt + 1) * 128, m],
                            wob[:, m * HID + jb * 400 :
                                m * HID + (jb + 1) * 400],
                            start=(m == 0), stop=(m == NM - 1))
                        wo_st["m"] += 1
                        if wo_st["m"] == NM:
                            ysb = pyb.tile([128, 400], F32, tag="ysb",
                                           name="ysb")
                            nc.vector.tensor_copy(out=ysb,
                                                  in_=wo_st["pys"][:, 0:400])
                            nc.sync.dma_start(
                                out=y.ap()[rt * 128 : (rt + 1) * 128,
                                           jb * 400 : (jb + 1) * 400],
                                in_=ysb)
                            wo_st["chain"] = None

                for h in range(HL):
                    if h >= 1:
                        # absorb the DMA waits of this head's prefetched q/k
                        _absorb(nc, qt[h])
                        _absorb(nc, kt[h])
                    if 1 <= h + 1 < HL:
                        # prefetch next head's q/k from DRAM spill
                        qk_tiles(h + 1)
                        for dst, src in ((qt[h + 1], qsp), (kt[h + 1], ksp)):
                            nc.sync.dma_start(
                                out=dst, in_=src.ap()[h * D : (h + 1) * D, :])
                    for sh in range(2):
                        s0 = sh * 1024
                        po = pso.tile([D + 1, 1024], F32, tag="po", name="po")
                        # software-pipelined: PV(tt-2) emitted after scores(tt)
                        LAG = 2
                        pts = [None] * TT
                        for tt in range(TT + LAG):
                            if tt < TT:
                                ss = psa.tile([128, 1024], F32, tag="ss",
                                              name="ss")
                                for sbb in range(2):
                                    _mm(nc, ss[:, sbb * 512 : (sbb + 1) * 512],
                                        kt[h][:, tt * 128 : (tt + 1) * 128],
                                        qt[h][:, s0 + sbb * 512 :
                                              s0 + (sbb + 1) * 512],
                                        start=True, stop=True)
                                wo_slot(1)
                                pt = ppt.tile([128, 1024], F32R, tag="pt",
                                              name="pt", bufs=LAG + 2)
                                nc.scalar.activation(
                                    out=pt, in_=ss[:, :], func=EXP,
                                    bias=bias_sb[:, h * TT + tt :
                                                 h * TT + tt + 1],
                                    scale=scale)
                                pts[tt] = pt
                            if tt >= LAG:
                                for sbb in range(2):
                                    _mm(nc, po[:, sbb * 512 : (sbb + 1) * 512],
                                        vp[:, (tt - LAG) * VW + h * 101 :
                                           (tt - LAG) * VW + h * 101 + 101],
                                        pts[tt - LAG][:, sbb * 512 :
                                                      (sbb + 1) * 512],
                                        start=(tt == LAG),
                                        stop=(tt == TT + LAG - 1))
                                pts[tt - LAG] = None
                                wo_slot(1)
                        # ---- on-chip normalization + eviction to ot ----
                        orow = por.tile([D + 1, 1024], F32R, tag="orow",
                                        name="orow")
                        nc.vector.tensor_copy(out=orow, in_=po[:, :])
                        # sums row (partition 100) -> partition 0 via DMA
                        srow = pnr.tile([1, 1024], F32R, tag="srow",
                                        name="srow")
                        rrow = pnr.tile([1, 1024], F32R, tag="rrow", name="rrow")
                        nc.sync.dma_start(out=srow, in_=orow[D : D + 1, :])
                        with nc.allow_low_precision(reason="softmax recip"):
                            nc.vector.reciprocal(out=rrow, in_=srow)
                        for blk in range(2):
                            pb = psw.tile([128, 512], F32, tag="nrm",
                                          name="pb", bufs=1)
                            _mm(nc, pb[0:D, :], ones1[0:1, :],
                                rrow[0:1, blk * 512 : (blk + 1) * 512],
                                start=True, stop=True)
                            nc.vector.tensor_mul(
                                ot[:, h * S + s0 + blk * 512 :
                                   h * S + s0 + (blk + 1) * 512],
                                orow[0:D, blk * 512 : (blk + 1) * 512],
                                pb[0:D, :])
                    # head h normalized -> unlock rts
                    for rt in range(8):
                        if RT_LAST_HEAD[rt] == h:
                            for jb in range(3):
                                wo_st["queue"].append((rt, jb))
                while wo_st["queue"] or wo_st["chain"] is not None:
                    wo_slot(1)


def make_core_inputs(x, alibi, attention_mask, wq, wk, wv, wo, layer_index):
    li = int(np.asarray(layer_index))
    inv = np.float32(1.0 / (li + 1))
    woT = np.ascontiguousarray(
        np.asarray(wo, dtype=np.float32).T).astype(ml_dtypes.bfloat16)
    xTs = [np.ascontiguousarray(np.asarray(x[b], dtype=np.float32).T)
           for b in range(B)]
    wts = []
    for g in range(HG):
        sl = slice(g * HL * D, (g + 1) * HL * D)
        wts.append(tuple(
            np.ascontiguousarray(np.asarray(w, dtype=np.float32)[sl, :].T)
            for w in (wq, wk, wv)))
    in_maps = []
    for b in range(B):
        for g in range(HG):
            a = np.asarray(alibi, dtype=np.float32)[
                b * H + g * HL : b * H + (g + 1) * HL, 0, :]      # (6, S)
            msk = np.asarray(attention_mask, dtype=np.float32)[b, 0, 0, :S]
            bias = a * inv + msk[None, :]                          # (6, S)
            biasT = np.ascontiguousarray(
                bias.reshape(HL, TT, 128).transpose(2, 0, 1).reshape(128, HL * TT))
            wqT, wkT, wvT = wts[g]
            in_maps.append({
                "xT": xTs[b], "wqT": wqT, "wkT": wkT, "wvT": wvT,
                "woT": woT, "biasT": biasT,
            })
    scale = float(np.float32(np.sqrt(np.float32(D))) * inv)
    return in_maps, scale


def run(trace=False, **inputs):
    in_maps, scale = make_core_inputs(**inputs)
    nc = build_program(scale)
    res = run_bass_kernel_spmd(nc, in_maps, core_ids=list(range(B * HG)),
                               trace=trace)
    out = np.empty((B, S, HID), dtype=np.float32)
    for b in range(B):
        for g in range(HG):
            out[b, g * ROWS : (g + 1) * ROWS, :] = res.results[b * HG + g]["y"]
    return out, res


def kernel(**inputs) -> np.ndarray:
    out, _ = run(trace=False, **inputs)
    return out
